# revision 14
# baseline (speedup 1.0000x reference)
"""Trainium2 Bass kernel for nn_NodeAggregator (gnn message passing / diffpool-style).

Reference math (per batch element b, forward pass only):
    h      = relu(x @ W1 + b1)                      [N, K]
    logits = h @ W2 + b2 + (-1e9)*(1-mask)[:,None]  [N, K]
    S      = softmax(logits, axis=-1)               [N, K]
    pfeat  = S.T @ x                                [K, F]
    pooled = S.T @ adj @ S                          [K, K]
    (threshold/topk/scatter + straight-through estimator is an exact
     no-op in the forward pass: a_sp + (pooled - a_sp) == pooled)
    d      = 1/sqrt(pooled.sum(-1) + 1e-9)
    padj   = pooled * d[:,None] * d[None,:]
    pmask  = ones

Sharding: data-parallel over batch B=8 across the 8 NeuronCores (one batch
element per core); weights replicated; no collectives.

Layout: everything stays in natural (row-major) orientation.  The big
contraction is computed as T = S.T @ adj (S stationary, adj the 512-wide
moving operand), then T is PE-transposed tile-by-tile into TT so that
pooled = TT.T @ S needs no further data movement.  x is passed
pre-transposed from the host (xT) for the h-stage, whose contraction runs
over F.  All inputs are host-retiled so that every DMA is a fully
contiguous per-partition transfer.

dtypes: matmul inputs in bf16 with fp32 PSUM accumulation; softmax and the
degree renormalization in fp32.  Measured ~2.6e-3 max relative error vs
the fp32 reference.  NK_F32=1 switches the MLP/softmax/pfeat path to
float32r (fp32-exact PE mode, ~4e-4 total error, ~25% slower).
"""

import os
from contextlib import ExitStack

import ml_dtypes
import numpy as np

import concourse.bass as bass
import concourse.tile as tile
from concourse import bacc, mybir
from concourse.masks import make_identity
from concourse.bass_utils import run_bass_kernel_spmd

B, N, F, K = 8, 2048, 512, 256
P = 128
NT = N // P   # 16 n-tiles
FT = F // P   # 4 f-tiles
KH = K // P   # 2 k-halves
NCH = 4       # xT n-chunks for the h-stage (512 wide)
CH = N // NCH
XC = 2        # x chunks
MH = 2        # adj column halves
MHW = N // MH
MC = MHW // 512
NCP = 19      # packed per-partition consts: b1(2) mb(16) eps(1)

F32 = mybir.dt.float32
F32R = mybir.dt.float32r
BF16 = mybir.dt.bfloat16
X = mybir.AxisListType
AF = mybir.ActivationFunctionType

F32_MODE = os.environ.get("NK_F32", "0") == "1"
DT_M = F32R if F32_MODE else BF16            # MLP/pfeat matmul dtype
NP_M = np.float32 if F32_MODE else ml_dtypes.bfloat16
DT_A = BF16                                   # adj-contraction dtype
NP_A = ml_dtypes.bfloat16


def _build_kernel(ctx: ExitStack, tc: tile.TileContext, io: dict, use_b2: bool, use_mask: bool):
    nc = tc.nc

    consts = ctx.enter_context(tc.tile_pool(name="consts", bufs=1))
    big = ctx.enter_context(tc.tile_pool(name="big", bufs=1))
    apool = ctx.enter_context(tc.tile_pool(name="apool", bufs=8))
    sm = ctx.enter_context(tc.tile_pool(name="sm", bufs=4))
    work = ctx.enter_context(tc.tile_pool(name="work", bufs=3))
    evp = ctx.enter_context(tc.tile_pool(name="evp", bufs=3))
    psum = ctx.enter_context(tc.tile_pool(name="psum", bufs=2, space="PSUM"))
    psum_tt = ctx.enter_context(tc.tile_pool(name="psum_tt", bufs=1, space="PSUM"))

    # ---- resident tensors / constants (DMA order = priority order) ----
    xT_sb = big.tile([P, FT, N], DT_M)
    w1_sb = consts.tile([P, FT, K], DT_M)
    for ft in range(FT):
        nc.sync.dma_start(w1_sb[:, ft, :], io["w1"][:, ft, :])
        nc.sync.dma_start(xT_sb[:, ft, 0:CH], io["xT"][0][:, ft, :])
    cp_sb = consts.tile([P, NCP], F32)
    nc.sync.dma_start(cp_sb, io["cpack"])
    b1_sb = cp_sb[:, 0:KH]
    mb_sb = cp_sb[:, KH:KH + NT]
    eps_sb = cp_sb[:, KH + NT:KH + NT + 1]
    for c in range(1, NCH):
        nc.sync.dma_start(xT_sb[:, :, c * CH:(c + 1) * CH], io["xT"][c])
    w2_sb = consts.tile([P, KH, K], DT_M)
    nc.sync.dma_start(w2_sb, io["w2"])
    if use_b2:
        b2b_sb = consts.tile([P, K], F32)
        nc.sync.dma_start(b2b_sb, io["b2"].partition_broadcast(P))
    # prime the ACT exp table while the input DMAs stream
    prime = work.tile([1, 1], F32, name="prime")
    nc.scalar.activation(prime, eps_sb[:1, :], AF.Exp)

    ident_sb = consts.tile([P, P], DT_A)
    make_identity(nc, ident_sb)
    d_sb = consts.tile([1, K], F32R)

    hT_sb = big.tile([P, KH, N], DT_M)
    S_sb = big.tile([P, NT, K], DT_M) if F32_MODE else None
    Sa_sb = big.tile([P, NT, K], DT_A)   # S in the adj-contraction dtype
    T_sb = big.tile([P, KH, N], DT_A)    # S.T @ adj, k on partitions
    TT_sb = big.tile([P, NT, K], DT_A)   # its transpose, m on partitions
    x_sb = big.tile([P, NT, F], DT_M)    # loaded on the 2nd HWDGE queue later
    Sp_sb = S_sb if F32_MODE else Sa_sb  # pfeat stationary operand

    # ---- phase 1: hT[k, n] = relu(W1.T @ xT + b1), n-chunk outer so the
    # softmax / adj pipeline can start early ----
    for nch in range(NCH):
        for kh in range(KH):
            ps = psum.tile([P, CH], F32, name="acc")
            for ft in range(FT):
                nc.tensor.matmul(
                    ps,
                    w1_sb[:, ft, kh * P:(kh + 1) * P],
                    xT_sb[:, ft, nch * CH:(nch + 1) * CH],
                    start=(ft == 0),
                    stop=(ft == FT - 1),
                )
            nc.scalar.activation(
                hT_sb[:, kh, nch * CH:(nch + 1) * CH], ps, AF.Relu,
                bias=b1_sb[:, kh:kh + 1], scale=1.0,
            )

    # ---- phase 2: logits = hT.T @ W2 + b2; S = softmax(logits + maskbias) ----
    for nt in range(NT):
        lp = psum.tile([P, K], F32, name="lg")
        for kh in range(KH):
            nc.tensor.matmul(
                lp,
                hT_sb[:, kh, nt * P:(nt + 1) * P],
                w2_sb[:, kh, :],
                start=(kh == 0),
                stop=(kh == KH - 1),
            )
        if use_b2:
            lg = work.tile([P, K], F32, name="lg_sb")
            nc.vector.tensor_add(lg, lp, b2b_sb)
        else:
            lg = lp
        if use_mask:
            mx = sm.tile([P, 1], F32, name="mx")
            nc.vector.reduce_max(mx, lg, axis=X.X)
            eb = sm.tile([P, 1], F32, name="eb")
            nc.vector.tensor_sub(eb, mb_sb[:, nt:nt + 1], mx)  # maskbias - max
        else:
            eb = sm.tile([P, 1], F32, name="eb")
            nc.vector.reduce_max(eb, lg, axis=X.X, negate=True)  # -max
        ex = work.tile([P, K], F32, name="ex")
        ssum = sm.tile([P, 1], F32, name="ssum")
        nc.scalar.activation(ex, lg, AF.Exp, bias=eb, scale=1.0, accum_out=ssum)
        rs = sm.tile([P, 1], F32, name="rs")
        nc.vector.reciprocal(rs, ssum)
        if F32_MODE:
            nc.vector.tensor_scalar_mul(S_sb[:, nt, :], ex, rs)
            nc.vector.tensor_copy(Sa_sb[:, nt, :], S_sb[:, nt, :])
        else:
            nc.vector.tensor_scalar_mul(Sa_sb[:, nt, :], ex, rs)

    # ---- phase 3: T[k, m] = S.T @ adj with S stationary and adj as the
    # wide moving operand, streamed in 1024-column half-slabs ----
    for mh in range(MH):
        tacc = {
            (kh, mc): psum_tt.tile([P, 512], F32, name=f"T{kh}{mc}")
            for kh in range(KH)
            for mc in range(MC)
        }
        for nt in range(NT):
            at = apool.tile([P, MHW], DT_A, name="aslab")
            nc.sync.dma_start(at, io["adj"][mh, nt])
            if mh == 0 and nt % 8 == 1:
                c = nt // 8
                nc.scalar.dma_start(x_sb[:, c * 8:(c + 1) * 8, :], io["x"][c])
            for kh in range(KH):
                for mc in range(MC):
                    nc.tensor.matmul(
                        tacc[(kh, mc)],
                        Sa_sb[:, nt, kh * P:(kh + 1) * P],
                        at[:, mc * 512:(mc + 1) * 512],
                        start=(nt == 0),
                        stop=(nt == NT - 1),
                    )
        for kh in range(KH):
            for mc in range(MC):
                nc.vector.tensor_copy(
                    T_sb[:, kh, mh * MHW + mc * 512:mh * MHW + (mc + 1) * 512],
                    tacc[(kh, mc)],
                )

    # ---- phase 4: transpose T -> TT (PE transpose, one 128x128 block each) ----
    for mt in range(NT):
        for kh in range(KH):
            tp = psum.tile([P, K], DT_A, name="lg")
            nc.tensor.transpose(
                tp[:, :P], T_sb[:, kh, mt * P:(mt + 1) * P], ident_sb
            )
            nc.vector.tensor_copy(TT_sb[:, mt, kh * P:(kh + 1) * P], tp[:, :P])

    # ---- phase 5: pfeat = S.T @ x ----
    for kh in range(KH):
        ps = psum.tile([P, F], F32, name="acc")
        for nt in range(NT):
            nc.tensor.matmul(
                ps,
                Sp_sb[:, nt, kh * P:(kh + 1) * P],
                x_sb[:, nt, :],
                start=(nt == 0),
                stop=(nt == NT - 1),
            )
        pe = evp.tile([P, F], F32, name="pf_ev")
        nc.vector.tensor_copy(pe, ps)
        nc.scalar.dma_start(io["pfeat"][kh * P:(kh + 1) * P, :], pe)

    # ---- phase 6: d = 1/sqrt(row_sum + eps) as a [1, K] row vector.
    # row_sum[k] = sum_l pooled[k,l] = sum_m T[k,m] (softmax rows sum to 1),
    # reduced on DVE then transposed per 128-half on the PE. ----
    identf_sb = consts.tile([P, P], F32)
    make_identity(nc, identf_sb)
    for kh in range(KH):
        rsv = sm.tile([P, 1], F32, name="rsv")
        nc.vector.reduce_sum(rsv, T_sb[:, kh, :], axis=X.X)
        dcol = sm.tile([P, 1], F32, name="dcol")
        nc.scalar.activation(dcol, rsv, AF.Sqrt, bias=eps_sb, scale=1.0)
        nc.vector.reciprocal(dcol, dcol)
        dt = psum.tile([P, K], F32, name="lg")
        nc.tensor.transpose(dt[:1, :P], dcol, identf_sb)
        nc.vector.tensor_copy(d_sb[:1, kh * P:(kh + 1) * P], dt[:1, :P])

    # ---- phase 7: pooled = TT.T @ S; padj = pooled * (d x d) ----
    dds = []
    for kh in range(KH):
        dd = psum.tile([P, 512], F32, name="acc")
        # dd[i, j] = d[kh*P+i] * d[j]  (outer product via K=1 matmul)
        nc.tensor.matmul(
            dd[:, :K], d_sb[:1, kh * P:(kh + 1) * P], d_sb[:1, :],
            start=True, stop=True,
        )
        dd_sb = evp.tile([P, K], F32, name="dd_sb")
        nc.vector.tensor_copy(dd_sb, dd[:, :K])
        dds.append(dd_sb)
    for kh in range(KH):
        pp = psum.tile([P, K], F32, name="lg")
        for mt in range(NT):
            nc.tensor.matmul(
                pp,
                TT_sb[:, mt, kh * P:(kh + 1) * P],
                Sa_sb[:, mt, :],
                start=(mt == 0),
                stop=(mt == NT - 1),
            )
        pa = evp.tile([P, K], F32, name="pa_ev")
        nc.vector.tensor_mul(pa, pp, dds[kh])
        nc.scalar.dma_start(io["padj"][kh * P:(kh + 1) * P, :], pa)


_CACHE = {}


def _get_nc(use_b2=False, use_mask=False):
    key = (use_b2, use_mask)
    if key in _CACHE:
        return _CACHE[key]
    nc = bacc.Bacc(
        "TRN2", target_bir_lowering=False, debug=False, enable_asserts=True
    )
    io = {
        "xT": nc.dram_tensor("xT", [NCH, P, FT, CH], DT_M, kind="ExternalInput").ap(),
        "x": nc.dram_tensor("x", [XC, P, NT // XC, F], DT_M, kind="ExternalInput").ap(),
        "adj": nc.dram_tensor("adj", [MH, NT, P, MHW], DT_A, kind="ExternalInput").ap(),
        "w1": nc.dram_tensor("w1", [P, FT, K], DT_M, kind="ExternalInput").ap(),
        "w2": nc.dram_tensor("w2", [P, KH, K], DT_M, kind="ExternalInput").ap(),
        "b2": nc.dram_tensor("b2", [K], F32, kind="ExternalInput").ap(),
        "cpack": nc.dram_tensor("cpack", [P, NCP], F32, kind="ExternalInput").ap(),
        "pfeat": nc.dram_tensor("pfeat", [K, F], F32, kind="ExternalOutput").ap(),
        "padj": nc.dram_tensor("padj", [K, K], F32, kind="ExternalOutput").ap(),
    }
    with tile.TileContext(nc) as tc, ExitStack() as ctx:
        _build_kernel(ctx, tc, io, use_b2, use_mask)
    nc.compile()
    _CACHE[key] = nc
    return nc


def make_in_maps(x, adj, mask, W1, b1, W2, b2):
    """Build the per-core input maps from the full (unsharded) inputs.

    All matmul operands are host-retiled into [chunk][partition][...]
    layouts so every device DMA is a fully contiguous per-partition read.
    """
    x = np.asarray(x, np.float32)
    adj = np.asarray(adj, np.float32)
    mask = np.asarray(mask, np.float32)
    w1 = np.asarray(W1, np.float32).astype(NP_M)
    w2 = np.asarray(W2, np.float32).astype(NP_M)
    b1v = np.asarray(b1, np.float32).reshape(K)
    b2v = np.ascontiguousarray(np.asarray(b2, np.float32).reshape(K))
    w1_t = np.ascontiguousarray(w1.reshape(FT, P, K).transpose(1, 0, 2))
    w2_t = np.ascontiguousarray(w2.reshape(KH, P, K).transpose(1, 0, 2))
    in_maps = []
    for b in range(B):
        xb = x[b]
        xm = xb.astype(NP_M)
        xT_t = np.ascontiguousarray(
            xm.T.reshape(FT, P, NCH, CH).transpose(2, 1, 0, 3)
        )
        x_t = np.ascontiguousarray(
            xm.reshape(XC, NT // XC, P, F).transpose(0, 2, 1, 3)
        )
        adj_t = np.ascontiguousarray(
            adj[b].astype(NP_A).reshape(NT, P, MH, MHW).transpose(2, 0, 1, 3)
        )
        mb = (-1e9 * (1.0 - mask[b])).astype(np.float32)
        cpack = np.empty((P, NCP), np.float32)
        cpack[:, 0] = b1v[0:P]
        cpack[:, 1] = b1v[P:2 * P]
        for t in range(NT):
            cpack[:, KH + t] = mb[t * P:(t + 1) * P]
        cpack[:, KH + NT] = 1e-9
        in_maps.append({
            "xT": xT_t,
            "x": x_t,
            "adj": adj_t,
            "w1": w1_t,
            "w2": w2_t,
            "b2": b2v,
            "cpack": cpack,
        })
    return in_maps


def run(x, adj, mask, W1, b1, W2, b2, trace=False):
    use_b2 = bool(np.any(np.asarray(b2)))
    use_mask = not bool(np.all(np.asarray(mask) == 1.0))
    nc = _get_nc(use_b2, use_mask)
    in_maps = make_in_maps(x, adj, mask, W1, b1, W2, b2)
    res = run_bass_kernel_spmd(nc, in_maps, core_ids=list(range(B)), trace=trace)
    pfeat = np.stack([res.results[b]["pfeat"] for b in range(B)]).astype(np.float32)
    padj = np.stack([res.results[b]["padj"] for b in range(B)]).astype(np.float32)
    pmask = np.ones((B, K), np.float32)
    return (pfeat, padj, pmask), res


def kernel(x, adj, mask, W1, b1, W2, b2):
    out, _ = run(x, adj, mask, W1, b1, W2, b2, trace=False)
    return out


# revision 15
# speedup vs baseline: 1.0581x; 1.0581x over previous
"""Trainium2 Bass kernel for nn_NodeAggregator (gnn message passing / diffpool-style).

Reference math (per batch element b, forward pass only):
    h      = relu(x @ W1 + b1)                      [N, K]
    logits = h @ W2 + b2 + (-1e9)*(1-mask)[:,None]  [N, K]
    S      = softmax(logits, axis=-1)               [N, K]
    pfeat  = S.T @ x                                [K, F]
    pooled = S.T @ adj @ S                          [K, K]
    (threshold/topk/scatter + straight-through estimator is an exact
     no-op in the forward pass: a_sp + (pooled - a_sp) == pooled)
    d      = 1/sqrt(pooled.sum(-1) + 1e-9)
    padj   = pooled * d[:,None] * d[None,:]
    pmask  = ones

Sharding: data-parallel over batch B=8 across the 8 NeuronCores (one batch
element per core); weights replicated; no collectives.

Layout: everything stays in natural (row-major) orientation.  The big
contraction is computed as T = S.T @ adj (S stationary, adj the 512-wide
moving operand), then T is PE-transposed tile-by-tile into TT so that
pooled = TT.T @ S needs no further data movement.  x is passed
pre-transposed from the host (xT) for the h-stage, whose contraction runs
over F.  All inputs are host-retiled so that every DMA is a fully
contiguous per-partition transfer.

dtypes: matmul inputs in bf16 with fp32 PSUM accumulation; softmax and the
degree renormalization in fp32.  Measured ~2.6e-3 max relative error vs
the fp32 reference.  NK_F32=1 switches the MLP/softmax/pfeat path to
float32r (fp32-exact PE mode, ~4e-4 total error, ~25% slower).
"""

import os
from contextlib import ExitStack

import ml_dtypes
import numpy as np

import concourse.bass as bass
import concourse.tile as tile
from concourse import bacc, mybir
from concourse.masks import make_identity
from concourse.bass_utils import run_bass_kernel_spmd

B, N, F, K = 8, 2048, 512, 256
P = 128
NT = N // P   # 16 n-tiles
FT = F // P   # 4 f-tiles
KH = K // P   # 2 k-halves
NCH = 4       # xT n-chunks for the h-stage (512 wide)
CH = N // NCH
XC = 2        # x chunks
MH = 2        # adj column halves
MHW = N // MH
MC = MHW // 512
NCP = 19      # packed per-partition consts: b1(2) mb(16) eps(1)

F32 = mybir.dt.float32
F32R = mybir.dt.float32r
BF16 = mybir.dt.bfloat16
X = mybir.AxisListType
AF = mybir.ActivationFunctionType

F32_MODE = os.environ.get("NK_F32", "0") == "1"
DT_M = F32R if F32_MODE else BF16            # MLP/pfeat matmul dtype
NP_M = np.float32 if F32_MODE else ml_dtypes.bfloat16
DT_A = BF16                                   # adj-contraction dtype
NP_A = ml_dtypes.bfloat16


def _build_kernel(ctx: ExitStack, tc: tile.TileContext, io: dict, use_b2: bool, use_mask: bool):
    nc = tc.nc

    consts = ctx.enter_context(tc.tile_pool(name="consts", bufs=1))
    big = ctx.enter_context(tc.tile_pool(name="big", bufs=1))
    apool = ctx.enter_context(tc.tile_pool(name="apool", bufs=8))
    sm = ctx.enter_context(tc.tile_pool(name="sm", bufs=4))
    work = ctx.enter_context(tc.tile_pool(name="work", bufs=3))
    evp = ctx.enter_context(tc.tile_pool(name="evp", bufs=3))
    psum = ctx.enter_context(tc.tile_pool(name="psum", bufs=2, space="PSUM"))
    psum_tt = ctx.enter_context(tc.tile_pool(name="psum_tt", bufs=1, space="PSUM"))

    # ---- resident tensors / constants (DMA order = priority order) ----
    xT_sb = big.tile([P, FT, N], DT_M)
    w1_sb = consts.tile([P, FT, K], DT_M)
    nc.sync.dma_start(w1_sb[:, 0, :], io["w1"][:, 0, :])
    nc.sync.dma_start(xT_sb[:, 0, 0:CH], io["xT"][0][:, 0, :])
    nc.sync.dma_start(w1_sb[:, 1:, :], io["w1"][:, 1:, :])
    nc.sync.dma_start(xT_sb[:, 1:, 0:CH], io["xT"][0][:, 1:, :])
    cp_sb = consts.tile([P, NCP], F32)
    nc.sync.dma_start(cp_sb, io["cpack"])
    b1_sb = cp_sb[:, 0:KH]
    mb_sb = cp_sb[:, KH:KH + NT]
    eps_sb = cp_sb[:, KH + NT:KH + NT + 1]
    for c in range(1, NCH):
        nc.sync.dma_start(xT_sb[:, :, c * CH:(c + 1) * CH], io["xT"][c])
    w2_sb = consts.tile([P, KH, K], DT_M)
    nc.sync.dma_start(w2_sb, io["w2"])
    if use_b2:
        b2b_sb = consts.tile([P, K], F32)
        nc.sync.dma_start(b2b_sb, io["b2"].partition_broadcast(P))
    # prime the ACT exp table while the input DMAs stream
    prime = work.tile([1, 1], F32, name="prime")
    nc.scalar.activation(prime, eps_sb[:1, :], AF.Exp)

    ident_sb = consts.tile([P, P], DT_A)
    make_identity(nc, ident_sb)
    d_sb = consts.tile([1, K], F32R)

    hT_sb = big.tile([P, KH, N], DT_M)
    S_sb = big.tile([P, NT, K], DT_M) if F32_MODE else None
    Sa_sb = big.tile([P, NT, K], DT_A)   # S in the adj-contraction dtype
    T_sb = big.tile([P, KH, N], DT_A)    # S.T @ adj, k on partitions
    TT_sb = big.tile([P, NT, K], DT_A)   # its transpose, m on partitions
    x_sb = big.tile([P, NT, F], DT_M)    # loaded on the 2nd HWDGE queue later
    Sp_sb = S_sb if F32_MODE else Sa_sb  # pfeat stationary operand

    # ---- phase 1: hT[k, n] = relu(W1.T @ xT + b1), n-chunk outer so the
    # softmax / adj pipeline can start early ----
    for nch in range(NCH):
        for kh in range(KH):
            ps = psum.tile([P, CH], F32, name="acc")
            for ft in range(FT):
                nc.tensor.matmul(
                    ps,
                    w1_sb[:, ft, kh * P:(kh + 1) * P],
                    xT_sb[:, ft, nch * CH:(nch + 1) * CH],
                    start=(ft == 0),
                    stop=(ft == FT - 1),
                )
            nc.scalar.activation(
                hT_sb[:, kh, nch * CH:(nch + 1) * CH], ps, AF.Relu,
                bias=b1_sb[:, kh:kh + 1], scale=1.0,
            )

    # ---- phase 2: logits = hT.T @ W2 + b2; S = softmax(logits + maskbias) ----
    for nt in range(NT):
        lp = psum.tile([P, K], F32, name="lg")
        for kh in range(KH):
            nc.tensor.matmul(
                lp,
                hT_sb[:, kh, nt * P:(nt + 1) * P],
                w2_sb[:, kh, :],
                start=(kh == 0),
                stop=(kh == KH - 1),
            )
        if use_b2:
            lg = work.tile([P, K], F32, name="lg_sb")
            nc.vector.tensor_add(lg, lp, b2b_sb)
        else:
            lg = lp
        if use_mask:
            mx = sm.tile([P, 1], F32, name="mx")
            nc.vector.reduce_max(mx, lg, axis=X.X)
            eb = sm.tile([P, 1], F32, name="eb")
            nc.vector.tensor_sub(eb, mb_sb[:, nt:nt + 1], mx)  # maskbias - max
        else:
            eb = sm.tile([P, 1], F32, name="eb")
            nc.vector.reduce_max(eb, lg, axis=X.X, negate=True)  # -max
        ex = work.tile([P, K], F32, name="ex")
        ssum = sm.tile([P, 1], F32, name="ssum")
        nc.scalar.activation(ex, lg, AF.Exp, bias=eb, scale=1.0, accum_out=ssum)
        rs = sm.tile([P, 1], F32, name="rs")
        nc.vector.reciprocal(rs, ssum)
        if F32_MODE:
            nc.vector.tensor_scalar_mul(S_sb[:, nt, :], ex, rs)
            nc.vector.tensor_copy(Sa_sb[:, nt, :], S_sb[:, nt, :])
        else:
            nc.vector.tensor_scalar_mul(Sa_sb[:, nt, :], ex, rs)

    # ---- phase 3: T[k, m] = S.T @ adj with S stationary and adj as the
    # wide moving operand, streamed in 1024-column half-slabs ----
    for mh in range(MH):
        tacc = {
            (kh, mc): psum_tt.tile([P, 512], F32, name=f"T{kh}{mc}")
            for kh in range(KH)
            for mc in range(MC)
        }
        for nt in range(NT):
            at = apool.tile([P, MHW], DT_A, name="aslab")
            nc.sync.dma_start(at, io["adj"][mh, nt])
            if mh == 0 and nt % 8 == 1:
                c = nt // 8
                nc.scalar.dma_start(x_sb[:, c * 8:(c + 1) * 8, :], io["x"][c])
            for kh in range(KH):
                for mc in range(MC):
                    nc.tensor.matmul(
                        tacc[(kh, mc)],
                        Sa_sb[:, nt, kh * P:(kh + 1) * P],
                        at[:, mc * 512:(mc + 1) * 512],
                        start=(nt == 0),
                        stop=(nt == NT - 1),
                    )
        for kh in range(KH):
            for mc in range(MC):
                nc.vector.tensor_copy(
                    T_sb[:, kh, mh * MHW + mc * 512:mh * MHW + (mc + 1) * 512],
                    tacc[(kh, mc)],
                )

    # ---- phase 4: transpose T -> TT (PE transpose, one 128x128 block each) ----
    for mt in range(NT):
        for kh in range(KH):
            tp = psum.tile([P, K], DT_A, name="lg")
            nc.tensor.transpose(
                tp[:, :P], T_sb[:, kh, mt * P:(mt + 1) * P], ident_sb
            )
            nc.vector.tensor_copy(TT_sb[:, mt, kh * P:(kh + 1) * P], tp[:, :P])

    # ---- phase 5: pfeat = S.T @ x ----
    for kh in range(KH):
        ps = psum.tile([P, F], F32, name="acc")
        for nt in range(NT):
            nc.tensor.matmul(
                ps,
                Sp_sb[:, nt, kh * P:(kh + 1) * P],
                x_sb[:, nt, :],
                start=(nt == 0),
                stop=(nt == NT - 1),
            )
        pe = evp.tile([P, F], F32, name="pf_ev")
        nc.vector.tensor_copy(pe, ps)
        nc.scalar.dma_start(io["pfeat"][kh * P:(kh + 1) * P, :], pe)

    # ---- phase 6: d = 1/sqrt(row_sum + eps) as a [1, K] row vector.
    # row_sum[k] = sum_l pooled[k,l] = sum_m T[k,m] (softmax rows sum to 1),
    # reduced on DVE then transposed per 128-half on the PE. ----
    identf_sb = consts.tile([P, P], F32)
    make_identity(nc, identf_sb)
    for kh in range(KH):
        rsv = sm.tile([P, 1], F32, name="rsv")
        nc.vector.reduce_sum(rsv, T_sb[:, kh, :], axis=X.X)
        dcol = sm.tile([P, 1], F32, name="dcol")
        nc.scalar.activation(dcol, rsv, AF.Sqrt, bias=eps_sb, scale=1.0)
        nc.vector.reciprocal(dcol, dcol)
        dt = psum.tile([P, K], F32, name="lg")
        nc.tensor.transpose(dt[:1, :P], dcol, identf_sb)
        nc.vector.tensor_copy(d_sb[:1, kh * P:(kh + 1) * P], dt[:1, :P])

    # ---- phase 7: pooled = TT.T @ S; padj = pooled * (d x d) ----
    dds = []
    for kh in range(KH):
        dd = psum.tile([P, 512], F32, name="acc")
        # dd[i, j] = d[kh*P+i] * d[j]  (outer product via K=1 matmul)
        nc.tensor.matmul(
            dd[:, :K], d_sb[:1, kh * P:(kh + 1) * P], d_sb[:1, :],
            start=True, stop=True,
        )
        dd_sb = evp.tile([P, K], F32, name="dd_sb")
        nc.vector.tensor_copy(dd_sb, dd[:, :K])
        dds.append(dd_sb)
    for kh in range(KH):
        pp = psum.tile([P, K], F32, name="lg")
        for mt in range(NT):
            nc.tensor.matmul(
                pp,
                TT_sb[:, mt, kh * P:(kh + 1) * P],
                Sa_sb[:, mt, :],
                start=(mt == 0),
                stop=(mt == NT - 1),
            )
        pa = evp.tile([P, K], F32, name="pa_ev")
        nc.vector.tensor_mul(pa, pp, dds[kh])
        nc.scalar.dma_start(io["padj"][kh * P:(kh + 1) * P, :], pa)


_CACHE = {}


def _get_nc(use_b2=False, use_mask=False):
    key = (use_b2, use_mask)
    if key in _CACHE:
        return _CACHE[key]
    nc = bacc.Bacc(
        "TRN2", target_bir_lowering=False, debug=False, enable_asserts=True
    )
    io = {
        "xT": nc.dram_tensor("xT", [NCH, P, FT, CH], DT_M, kind="ExternalInput").ap(),
        "x": nc.dram_tensor("x", [XC, P, NT // XC, F], DT_M, kind="ExternalInput").ap(),
        "adj": nc.dram_tensor("adj", [MH, NT, P, MHW], DT_A, kind="ExternalInput").ap(),
        "w1": nc.dram_tensor("w1", [P, FT, K], DT_M, kind="ExternalInput").ap(),
        "w2": nc.dram_tensor("w2", [P, KH, K], DT_M, kind="ExternalInput").ap(),
        "b2": nc.dram_tensor("b2", [K], F32, kind="ExternalInput").ap(),
        "cpack": nc.dram_tensor("cpack", [P, NCP], F32, kind="ExternalInput").ap(),
        "pfeat": nc.dram_tensor("pfeat", [K, F], F32, kind="ExternalOutput").ap(),
        "padj": nc.dram_tensor("padj", [K, K], F32, kind="ExternalOutput").ap(),
    }
    with tile.TileContext(nc) as tc, ExitStack() as ctx:
        _build_kernel(ctx, tc, io, use_b2, use_mask)
    nc.compile()
    _CACHE[key] = nc
    return nc


def make_in_maps(x, adj, mask, W1, b1, W2, b2):
    """Build the per-core input maps from the full (unsharded) inputs.

    All matmul operands are host-retiled into [chunk][partition][...]
    layouts so every device DMA is a fully contiguous per-partition read.
    """
    x = np.asarray(x, np.float32)
    adj = np.asarray(adj, np.float32)
    mask = np.asarray(mask, np.float32)
    w1 = np.asarray(W1, np.float32).astype(NP_M)
    w2 = np.asarray(W2, np.float32).astype(NP_M)
    b1v = np.asarray(b1, np.float32).reshape(K)
    b2v = np.ascontiguousarray(np.asarray(b2, np.float32).reshape(K))
    w1_t = np.ascontiguousarray(w1.reshape(FT, P, K).transpose(1, 0, 2))
    w2_t = np.ascontiguousarray(w2.reshape(KH, P, K).transpose(1, 0, 2))
    in_maps = []
    for b in range(B):
        xb = x[b]
        xm = xb.astype(NP_M)
        xT_t = np.ascontiguousarray(
            xm.T.reshape(FT, P, NCH, CH).transpose(2, 1, 0, 3)
        )
        x_t = np.ascontiguousarray(
            xm.reshape(XC, NT // XC, P, F).transpose(0, 2, 1, 3)
        )
        adj_t = np.ascontiguousarray(
            adj[b].astype(NP_A).reshape(NT, P, MH, MHW).transpose(2, 0, 1, 3)
        )
        mb = (-1e9 * (1.0 - mask[b])).astype(np.float32)
        cpack = np.empty((P, NCP), np.float32)
        cpack[:, 0] = b1v[0:P]
        cpack[:, 1] = b1v[P:2 * P]
        for t in range(NT):
            cpack[:, KH + t] = mb[t * P:(t + 1) * P]
        cpack[:, KH + NT] = 1e-9
        in_maps.append({
            "xT": xT_t,
            "x": x_t,
            "adj": adj_t,
            "w1": w1_t,
            "w2": w2_t,
            "b2": b2v,
            "cpack": cpack,
        })
    return in_maps


def run(x, adj, mask, W1, b1, W2, b2, trace=False):
    use_b2 = bool(np.any(np.asarray(b2)))
    use_mask = not bool(np.all(np.asarray(mask) == 1.0))
    nc = _get_nc(use_b2, use_mask)
    in_maps = make_in_maps(x, adj, mask, W1, b1, W2, b2)
    res = run_bass_kernel_spmd(nc, in_maps, core_ids=list(range(B)), trace=trace)
    pfeat = np.stack([res.results[b]["pfeat"] for b in range(B)]).astype(np.float32)
    padj = np.stack([res.results[b]["padj"] for b in range(B)]).astype(np.float32)
    pmask = np.ones((B, K), np.float32)
    return (pfeat, padj, pmask), res


def kernel(x, adj, mask, W1, b1, W2, b2):
    out, _ = run(x, adj, mask, W1, b1, W2, b2, trace=False)
    return out
